# revision 24
# baseline (speedup 1.0000x reference)
"""Masked multi-head self-attention (sparse_attention) on 8 Trainium2 cores.

Strategy (v3)
-------------
Shard the fused (batch*heads)=16 leading dim of q/k/v across 8 cores, 2 heads
per core.  Per head the kernel computes S^T = K''@Q''^T in [j, i] orientation
on the tensor engine, where Q''/K'' carry two extra contraction rows that
encode the bbox mask additively: q''[80]=M*mA_i, q''[81]=M*mB_i and
k''[80]=-M*mB_j, k''[81]=-M*mA_j, so blocked (i,j) pairs get -M^2 added to
the score and fall out of both exp paths naturally.  No key sorting, no
accumulator groups, no combine pass.

Scores are produced pre-scaled into fp16-Schraudolph bit space:
t = A_h*u where u = q.k/sqrt(dh) and A_h = 1024/ln2.  The exp(u - C) of each
[128, 1024] score pair-tile is then evaluated on ONE of TWO engines in
parallel (static assignment):
  - ACT pairs (9/16): scalar-engine exp (scale=1/A_h, bias=-C) -> fp16 P.
  - DVE pairs (7/16): one vector-engine tensor_scalar (add IMM, max 0) ->
    int16 whose bit pattern IS fp16(exp(u-C)) (Schraudolph, ~3% rel err).
Both feed plain fp16 PV matmuls accumulating into one [112, 512] PSUM tile
per i-block; softmax denominators fall out of a ones-column in V.  Per
i-block the accumulator is copied to SBUF and DMA'd out unnormalized; the
host divides by the sums row, transposes, and reassembles heads.  Inputs
stream on two DMA queues (sync: q/k, gpsimd: v + outputs).
"""

import math
import os

import numpy as np

N_CORES = 8
P = 128  # partitions / j-chunk rows
IB = 512  # i-block width (psum bank, fp32)
DH = 80  # head dim
DV = 97  # V cols: 80 values + sum row at 96
SUM_ROW = 96
MASK = 192.0  # mask row magnitude; blocked scores get -MASK^2
C_SHIFT = 4.0  # global exp shift (range headroom; cancels in softmax)
MU = 0.044  # Schraudolph bias tuning
A_H = 1024.0 / math.log(2.0)
IMM = 15360.0 - 1024.0 * MU - C_SHIFT * A_H

# pair-tile indices handled by the vector engine (rest go to scalar engine);
# strict alternation keeps each exp engine ahead of the PV matmuls
DVE_PAIRS = (1, 3, 5, 7, 9, 11, 13)

_PROGRAM_CACHE = {}
LAST_RESULTS = None  # BassKernelResults of the most recent run (for test.py)


def _subject_masks_np(bboxes: np.ndarray, resolution: int) -> np.ndarray:
    b = bboxes[0].astype(np.float32)  # [s, 4]
    x0 = np.round(b[:, 0] * resolution)
    y0 = np.round(b[:, 1] * resolution)
    x1 = np.round(b[:, 2] * resolution)
    y1 = np.round(b[:, 3] * resolution)
    coords = np.arange(resolution, dtype=np.float32)
    xm = (coords[None, :] >= x0[:, None]) & (coords[None, :] < x1[:, None])
    ym = (coords[None, :] >= y0[:, None]) & (coords[None, :] < y1[:, None])
    return (ym[:, :, None] & xm[:, None, :]).reshape(b.shape[0], -1)  # [s, n]


def _build_program(n, heads_per_core, dve_pairs):
    import concourse.mybir as mybir
    import concourse.tile as tile
    from concourse import bacc

    f32 = mybir.dt.float32
    f16 = mybir.dt.float16
    i16 = mybir.dt.int16
    Exp = mybir.ActivationFunctionType.Exp
    ADD = mybir.AluOpType.add
    MAX = mybir.AluOpType.max

    nch = n // P
    npair = nch // 2
    n_ib = n // IB

    nc = bacc.Bacc("TRN2", target_bir_lowering=False, debug=False,
                   num_devices=N_CORES)
    q_d = nc.dram_tensor("q16", [heads_per_core, P, n], f16,
                         kind="ExternalInput")
    k_d = nc.dram_tensor("k16", [heads_per_core, P, nch, P], f16,
                         kind="ExternalInput")
    v16_d = nc.dram_tensor("v16", [heads_per_core, P, nch, DV], f16,
                           kind="ExternalInput")
    o_d = nc.dram_tensor("o", [heads_per_core, n_ib, P, IB], f16,
                         kind="ExternalOutput")

    with tile.TileContext(nc) as tc:
        with (
            tc.tile_pool(name="const", bufs=1) as const_pool,
            tc.tile_pool(name="head", bufs=2) as head_pool,
            tc.tile_pool(name="p16", bufs=4) as p16_pool,
            tc.tile_pool(name="pb", bufs=4) as pb_pool,
            tc.tile_pool(name="out", bufs=8) as out_pool,
            tc.tile_pool(name="s_ps", bufs=3, space="PSUM") as s_pool,
            tc.tile_pool(name="acc_ps", bufs=2, space="PSUM") as acc_pool,
        ):
            bias_c = const_pool.tile([P, 1], f32)
            nc.vector.memset(bias_c[:], -C_SHIFT)

            # pre-warm the exp table set while the first DMAs run
            warm = const_pool.tile([P, 1], f32)
            nc.vector.memset(warm[:], 0.0)
            nc.scalar.activation(warm[:], warm[:], Exp)

            # PE warm-up: dependency-free matmuls on const data during the
            # DMA ramp so the HAM clock gate reaches 8/8 before real work
            warm_w = const_pool.tile([P, 256], f16)
            nc.vector.memset(warm_w[:], 0.0)
            for r in range(24):
                sw = s_pool.tile([P, 2 * IB], f32, tag="s", name=f"warm{r}")
                nc.tensor.matmul(sw[:, 0:256], lhsT=warm_w[:, 0:P],
                                 rhs=warm_w[:], start=True, stop=True)

            def load_head(h, first):
                q16 = head_pool.tile([P, n], f16, tag="q16", name=f"q16_{h}")
                k16 = head_pool.tile([P, nch, P], f16, tag="k16",
                                     name=f"k16_{h}")
                v16 = head_pool.tile([P, nch, DV], f16, tag="v16",
                                     name=f"v16_{h}")
                if first:
                    # head 0: bulk on the sync queue (16-engine DMA fan-out)
                    # in first-needed order; tail chunks on gpsimd in
                    # parallel so i-block 0 never waits
                    nc.gpsimd.dma_start(k16[:, 26:nch, :],
                                        k_d[h][:, 26:nch, :])
                    nc.sync.dma_start(q16[:, 0:IB], q_d[h][:, 0:IB])
                    nc.sync.dma_start(k16[:, 0:4, :], k_d[h][:, 0:4, :])
                    nc.sync.dma_start(v16[:, 0:4, :], v16_d[h][:, 0:4, :])
                    nc.gpsimd.dma_start(v16[:, 26:nch, :],
                                        v16_d[h][:, 26:nch, :])
                    nc.sync.dma_start(k16[:, 4:12, :], k_d[h][:, 4:12, :])
                    nc.sync.dma_start(v16[:, 4:12, :], v16_d[h][:, 4:12, :])
                    nc.sync.dma_start(k16[:, 12:20, :], k_d[h][:, 12:20, :])
                    nc.sync.dma_start(v16[:, 12:20, :],
                                      v16_d[h][:, 12:20, :])
                    nc.sync.dma_start(k16[:, 20:26, :], k_d[h][:, 20:26, :])
                    nc.sync.dma_start(v16[:, 20:26, :],
                                      v16_d[h][:, 20:26, :])
                    nc.sync.dma_start(q16[:, IB:2 * IB],
                                      q_d[h][:, IB:2 * IB])
                    nc.sync.dma_start(q16[:, 2 * IB:4 * IB],
                                      q_d[h][:, 2 * IB:4 * IB])
                    nc.sync.dma_start(q16[:, 4 * IB:n], q_d[h][:, 4 * IB:n])
                else:
                    nc.sync.dma_start(q16[:], q_d[h][:])
                    nc.sync.dma_start(k16[:], k_d[h][:])
                    nc.sync.dma_start(v16[:], v16_d[h][:])
                return q16, k16, v16

            head_tiles = {0: load_head(0, True)}
            pending_pvs = []
            pending_out = None

            for h in range(heads_per_core):
                if h not in head_tiles:
                    head_tiles[h] = load_head(h, False)
                q16, k16, v16 = head_tiles[h]

                for ib in range(n_ib):
                    if ib == 3 and h + 1 < heads_per_core \
                            and h + 1 not in head_tiles:
                        head_tiles[h + 1] = load_head(h + 1, False)
                    acc = acc_pool.tile([DV, IB], f32, tag="acc",
                                        name=f"acc_{h}_{ib}")
                    q_sl = q16[0:82, ib * IB:(ib + 1) * IB]
                    first = [True]

                    for t in range(npair):
                        c0, c1 = 2 * t, 2 * t + 1
                        s = s_pool.tile([P, 2 * IB], f32, tag="s")
                        nc.tensor.matmul(s[:, 0:IB], lhsT=k16[0:82, c0, :],
                                         rhs=q_sl, start=True, stop=True)
                        nc.tensor.matmul(s[:, IB:2 * IB],
                                         lhsT=k16[0:82, c1, :],
                                         rhs=q_sl, start=True, stop=True)

                        if t == npair - 1:
                            # split the block's last pair across BOTH exp
                            # engines so the final PV isn't waiting
                            p16 = p16_pool.tile([P, 2 * IB], f16, tag="p16")
                            pb = pb_pool.tile([P, 2 * IB], i16, tag="pb")
                            nc.scalar.activation(p16[:, 0:IB], s[:, 0:IB],
                                                 Exp, scale=float(1.0 / A_H),
                                                 bias=bias_c[:])
                            nc.vector.tensor_scalar(pb[:, IB:2 * IB],
                                                    s[:, IB:2 * IB], IMM,
                                                    0.0, op0=ADD, op1=MAX)
                            rhs0 = p16[:, 0:IB]
                            rhs1 = pb[:, IB:2 * IB].bitcast(f16)
                        elif t in dve_pairs:
                            pb = pb_pool.tile([P, 2 * IB], i16, tag="pb")
                            nc.vector.tensor_scalar(pb[:], s[:], IMM, 0.0,
                                                    op0=ADD, op1=MAX)
                            rhs0 = pb[:, 0:IB].bitcast(f16)
                            rhs1 = pb[:, IB:2 * IB].bitcast(f16)
                        else:
                            p16 = p16_pool.tile([P, 2 * IB], f16, tag="p16")
                            nc.scalar.activation(p16[:], s[:], Exp,
                                                 scale=float(1.0 / A_H),
                                                 bias=bias_c[:])
                            rhs0 = p16[:, 0:IB]
                            rhs1 = p16[:, IB:2 * IB]

                        def make_pv(rhs0=rhs0, rhs1=rhs1, c0=c0, c1=c1, t=t,
                                    acc=acc, v16=v16, first=first):
                            def pv():
                                nc.tensor.matmul(
                                    acc[:], lhsT=v16[:, c0, :], rhs=rhs0,
                                    start=first[0], stop=False)
                                first[0] = False
                                nc.tensor.matmul(
                                    acc[:], lhsT=v16[:, c1, :], rhs=rhs1,
                                    start=False, stop=(t == npair - 1))
                            return pv

                        pending_pvs.append(make_pv())
                        if len(pending_pvs) > 3:
                            pending_pvs.pop(0)()
                        if t == 4 and pending_out is not None:
                            pending_out()
                            pending_out = None

                    def make_out(acc=acc, h=h, ib=ib):
                        def out():
                            # 128-partition staging keeps the DMA engine
                            # fan-out balanced (rows 97..127 are junk)
                            o_sb = out_pool.tile([P, IB], f16, tag="osb",
                                                 name=f"o_{h}_{ib}")
                            nc.vector.tensor_copy(o_sb[0:97, :],
                                                  acc[0:97, :])
                            nc.sync.dma_start(o_d[h, ib], o_sb[:])
                        return out

                    if pending_out is not None:
                        pending_out()
                    pending_out = make_out()

            while pending_pvs:
                pending_pvs.pop(0)()
            if pending_out is not None:
                pending_out()

    nc.compile()
    return nc


def kernel(hidden_states, q, k, v, bboxes, is_cross, ith, num_heads):
    global LAST_RESULTS
    if is_cross:
        return np.asarray(hidden_states)

    from concourse.bass_utils import run_bass_kernel_spmd

    q = np.asarray(q, dtype=np.float32)
    k = np.asarray(k, dtype=np.float32)
    v = np.asarray(v, dtype=np.float32)
    bboxes = np.asarray(bboxes, dtype=np.float32)
    num_heads = int(num_heads)

    bh, n, dh = q.shape
    assert dh == DH and bh % N_CORES == 0 and n % IB == 0
    heads_per_core = bh // N_CORES
    batch = bh // num_heads
    nch = n // P
    n_ib = n // IB
    scale = 1.0 / math.sqrt(dh)
    g = math.sqrt(A_H * scale)

    res_sq = int(math.isqrt(n))
    subj = _subject_masks_np(bboxes, res_sq)
    assert subj.shape[0] == 2, "kernel specialized for 2 subject boxes"
    mA = (subj[0] & ~subj[1]).astype(np.float32)  # A-only
    mB = (subj[1] & ~subj[0]).astype(np.float32)  # B-only

    f16 = np.float16

    qT = q.transpose(0, 2, 1) * g  # [bh, 80, n]
    kT = k.transpose(0, 2, 1) * g
    q16 = np.zeros((bh, P, n), f16)
    q16[:, :DH] = qT
    q16[:, DH] = MASK * mA
    q16[:, DH + 1] = MASK * mB
    k16 = np.zeros((bh, P, n), f16)
    k16[:, :DH] = kT
    k16[:, DH] = -MASK * mB
    k16[:, DH + 1] = -MASK * mA
    k16 = k16.reshape(bh, P, nch, P)

    vt = np.zeros((bh, n, DV), f16)
    vt[:, :, :DH] = v
    vt[:, :, SUM_ROW] = 1.0
    v16 = np.ascontiguousarray(
        vt.reshape(bh, nch, P, DV).transpose(0, 2, 1, 3))

    key = (n, heads_per_core, DVE_PAIRS)
    if key not in _PROGRAM_CACHE:
        _PROGRAM_CACHE[key] = _build_program(n, heads_per_core,
                                             frozenset(DVE_PAIRS))
    nc = _PROGRAM_CACHE[key]

    in_maps = []
    for c in range(N_CORES):
        sl = slice(c * heads_per_core, (c + 1) * heads_per_core)
        in_maps.append({"q16": q16[sl], "k16": k16[sl], "v16": v16[sl]})

    trace = bool(int(os.environ.get("BASS_ATTN_TRACE", "0")))
    kwargs = {}
    if trace:
        kwargs = dict(trace=True, trace_cores=list(range(N_CORES)))
    res = run_bass_kernel_spmd(nc, in_maps, core_ids=list(range(N_CORES)),
                               **kwargs)
    LAST_RESULTS = res

    out = np.empty((batch, n, num_heads * dh), np.float32)
    for bh_idx in range(bh):
        c, hh = divmod(bh_idx, heads_per_core)
        b, hd = divmod(bh_idx, num_heads)
        o = res.results[c]["o"][hh][:, 0:97].astype(np.float32)
        den = o[:, SUM_ROW, :]  # [n_ib, IB]
        on = o[:, :DH, :] / den[:, None, :]  # [n_ib, 80, IB]
        out[b, :, hd * dh:(hd + 1) * dh] = (
            on.transpose(0, 2, 1).reshape(n, dh))
    return out


# revision 26
# speedup vs baseline: 1.0183x; 1.0183x over previous
"""Masked multi-head self-attention (sparse_attention) on 8 Trainium2 cores.

Strategy
--------
Shard the fused (batch*heads)=16 leading dim of q/k/v across 8 cores, 2 heads
per core.  Per head the kernel computes S^T = K''@Q''^T in [j, i] orientation
on the tensor engine, where Q''/K'' carry two extra contraction rows that
encode the bbox mask additively: q''[80]=M*mA_i, q''[81]=M*mB_i and
k''[80]=-M*mB_j, k''[81]=-M*mA_j, so blocked (i,j) pairs get -M^2 added to
the score and fall out of both exp paths naturally.  No key sorting, no
accumulator groups, no combine pass.

Scores are produced pre-scaled into fp16-Schraudolph bit space:
t = A_h*u where u = q.k/sqrt(dh) and A_h = 1024/ln2.  The exp(u - C) of each
[128, 1024] score pair-tile is then evaluated on ONE of TWO engines in
parallel (static assignment):
  - ACT pairs (9/16): scalar-engine exp (scale=1/A_h, bias=-C) -> fp16 P.
  - DVE pairs (7/16): one vector-engine tensor_scalar (add IMM, max 0) ->
    int16 whose bit pattern IS fp16(exp(u-C)) (Schraudolph, ~3% rel err).
Both feed plain fp16 PV matmuls accumulating into one [97, 512] PSUM tile
per i-block; softmax denominators fall out of a ones-column in V.  Per
i-block the accumulator is copied to SBUF and DMA'd out unnormalized; the
host divides by the sums row, transposes, and reassembles heads.

Scheduling notes: PV matmuls trail their exp by 3 pair-tiles (and the lag
carries across i-block boundaries) so the tensor engine never waits on the
exp engines; each block's final pair is split across both exp engines; q/k
are padded to 128 partitions purely so the DMA fans out evenly over all 16
DMA engines (82-partition transfers hot-spot 2 engines ~4x); a burst of
dependency-free warm-up matmuls during the input DMA ramp lifts the PE HAM
clock gate to 8/8 before real work arrives.
"""

import math
import os

import numpy as np

N_CORES = 8
P = 128  # partitions / j-chunk rows
IB = 512  # i-block width (psum bank, fp32)
DH = 80  # head dim
DV = 97  # V cols: 80 values + sum row at 96
SUM_ROW = 96
MASK = 192.0  # mask row magnitude; blocked scores get -MASK^2
C_SHIFT = 4.0  # global exp shift (range headroom; cancels in softmax)
MU = 0.044  # Schraudolph bias tuning
A_H = 1024.0 / math.log(2.0)
IMM = 15360.0 - 1024.0 * MU - C_SHIFT * A_H

# pair-tile indices handled by the vector engine (rest go to scalar engine);
# strict alternation keeps each exp engine ahead of the PV matmuls
DVE_PAIRS = (1, 3, 6, 9, 11, 13)

_PROGRAM_CACHE = {}
LAST_RESULTS = None  # BassKernelResults of the most recent run (for test.py)


def _subject_masks_np(bboxes: np.ndarray, resolution: int) -> np.ndarray:
    b = bboxes[0].astype(np.float32)  # [s, 4]
    x0 = np.round(b[:, 0] * resolution)
    y0 = np.round(b[:, 1] * resolution)
    x1 = np.round(b[:, 2] * resolution)
    y1 = np.round(b[:, 3] * resolution)
    coords = np.arange(resolution, dtype=np.float32)
    xm = (coords[None, :] >= x0[:, None]) & (coords[None, :] < x1[:, None])
    ym = (coords[None, :] >= y0[:, None]) & (coords[None, :] < y1[:, None])
    return (ym[:, :, None] & xm[:, None, :]).reshape(b.shape[0], -1)  # [s, n]


def _build_program(n, heads_per_core, dve_pairs):
    import concourse.mybir as mybir
    import concourse.tile as tile
    from concourse import bacc

    f32 = mybir.dt.float32
    f16 = mybir.dt.float16
    i16 = mybir.dt.int16
    Exp = mybir.ActivationFunctionType.Exp
    ADD = mybir.AluOpType.add
    MAX = mybir.AluOpType.max

    nch = n // P
    npair = nch // 2
    n_ib = n // IB

    nc = bacc.Bacc("TRN2", target_bir_lowering=False, debug=False,
                   num_devices=N_CORES)
    q_d = nc.dram_tensor("q16", [heads_per_core, P, n], f16,
                         kind="ExternalInput")
    k_d = nc.dram_tensor("k16", [heads_per_core, P, nch, P], f16,
                         kind="ExternalInput")
    v16_d = nc.dram_tensor("v16", [heads_per_core, P, nch, DV], f16,
                           kind="ExternalInput")
    o_d = nc.dram_tensor("o", [heads_per_core, n_ib, P, IB], f16,
                         kind="ExternalOutput")

    with tile.TileContext(nc) as tc:
        with (
            tc.tile_pool(name="const", bufs=1) as const_pool,
            tc.tile_pool(name="head", bufs=2) as head_pool,
            tc.tile_pool(name="p16", bufs=4) as p16_pool,
            tc.tile_pool(name="pb", bufs=4) as pb_pool,
            tc.tile_pool(name="out", bufs=8) as out_pool,
            tc.tile_pool(name="s_ps", bufs=3, space="PSUM") as s_pool,
            tc.tile_pool(name="acc_ps", bufs=2, space="PSUM") as acc_pool,
        ):
            bias_c = const_pool.tile([P, 1], f32)
            nc.vector.memset(bias_c[:], -C_SHIFT)

            # pre-warm the exp table set while the first DMAs run
            warm = const_pool.tile([P, 1], f32)
            nc.vector.memset(warm[:], 0.0)
            nc.scalar.activation(warm[:], warm[:], Exp)

            # PE warm-up: dependency-free matmuls on const data during the
            # DMA ramp so the HAM clock gate reaches 8/8 before real work
            warm_w = const_pool.tile([P, 256], f16)
            nc.vector.memset(warm_w[:], 0.0)
            for r in range(24):
                sw = s_pool.tile([P, 2 * IB], f32, tag="s", name=f"warm{r}")
                nc.tensor.matmul(sw[:, 0:256], lhsT=warm_w[:, 0:P],
                                 rhs=warm_w[:], start=True, stop=True)

            def load_head(h, first):
                q16 = head_pool.tile([P, n], f16, tag="q16", name=f"q16_{h}")
                k16 = head_pool.tile([P, nch, P], f16, tag="k16",
                                     name=f"k16_{h}")
                v16 = head_pool.tile([P, nch, DV], f16, tag="v16",
                                     name=f"v16_{h}")
                if first:
                    # head 0: bulk on the sync queue (16-engine DMA fan-out)
                    # in first-needed order; tail chunks on gpsimd in
                    # parallel so i-block 0 never waits
                    nc.gpsimd.dma_start(k16[:, 26:nch, :],
                                        k_d[h][:, 26:nch, :])
                    nc.sync.dma_start(q16[:, 0:IB], q_d[h][:, 0:IB])
                    nc.sync.dma_start(k16[:, 0:4, :], k_d[h][:, 0:4, :])
                    nc.sync.dma_start(v16[:, 0:4, :], v16_d[h][:, 0:4, :])
                    nc.gpsimd.dma_start(v16[:, 26:nch, :],
                                        v16_d[h][:, 26:nch, :])
                    nc.sync.dma_start(k16[:, 4:12, :], k_d[h][:, 4:12, :])
                    nc.sync.dma_start(v16[:, 4:12, :], v16_d[h][:, 4:12, :])
                    nc.sync.dma_start(k16[:, 12:20, :], k_d[h][:, 12:20, :])
                    nc.sync.dma_start(v16[:, 12:20, :],
                                      v16_d[h][:, 12:20, :])
                    nc.sync.dma_start(k16[:, 20:26, :], k_d[h][:, 20:26, :])
                    nc.sync.dma_start(v16[:, 20:26, :],
                                      v16_d[h][:, 20:26, :])
                    nc.sync.dma_start(q16[:, IB:2 * IB],
                                      q_d[h][:, IB:2 * IB])
                    nc.sync.dma_start(q16[:, 2 * IB:4 * IB],
                                      q_d[h][:, 2 * IB:4 * IB])
                    nc.sync.dma_start(q16[:, 4 * IB:n], q_d[h][:, 4 * IB:n])
                else:
                    nc.sync.dma_start(q16[:], q_d[h][:])
                    nc.sync.dma_start(k16[:], k_d[h][:])
                    nc.sync.dma_start(v16[:], v16_d[h][:])
                return q16, k16, v16

            head_tiles = {0: load_head(0, True)}
            pending_pvs = []
            pending_out = None

            for h in range(heads_per_core):
                if h not in head_tiles:
                    head_tiles[h] = load_head(h, False)
                q16, k16, v16 = head_tiles[h]

                for ib in range(n_ib):
                    if ib == 3 and h + 1 < heads_per_core \
                            and h + 1 not in head_tiles:
                        head_tiles[h + 1] = load_head(h + 1, False)
                    acc = acc_pool.tile([DV, IB], f32, tag="acc",
                                        name=f"acc_{h}_{ib}")
                    q_sl = q16[0:82, ib * IB:(ib + 1) * IB]
                    first = [True]

                    for t in range(npair):
                        c0, c1 = 2 * t, 2 * t + 1
                        s = s_pool.tile([P, 2 * IB], f32, tag="s")
                        nc.tensor.matmul(s[:, 0:IB], lhsT=k16[0:82, c0, :],
                                         rhs=q_sl, start=True, stop=True)
                        nc.tensor.matmul(s[:, IB:2 * IB],
                                         lhsT=k16[0:82, c1, :],
                                         rhs=q_sl, start=True, stop=True)

                        if t == npair - 1:
                            # split the block's last pair across BOTH exp
                            # engines so the final PV isn't waiting
                            p16 = p16_pool.tile([P, 2 * IB], f16, tag="p16")
                            pb = pb_pool.tile([P, 2 * IB], i16, tag="pb")
                            nc.scalar.activation(p16[:, 0:IB], s[:, 0:IB],
                                                 Exp, scale=float(1.0 / A_H),
                                                 bias=bias_c[:])
                            nc.vector.tensor_scalar(pb[:, IB:2 * IB],
                                                    s[:, IB:2 * IB], IMM,
                                                    0.0, op0=ADD, op1=MAX)
                            rhs0 = p16[:, 0:IB]
                            rhs1 = pb[:, IB:2 * IB].bitcast(f16)
                        elif t in dve_pairs:
                            pb = pb_pool.tile([P, 2 * IB], i16, tag="pb")
                            nc.vector.tensor_scalar(pb[:], s[:], IMM, 0.0,
                                                    op0=ADD, op1=MAX)
                            rhs0 = pb[:, 0:IB].bitcast(f16)
                            rhs1 = pb[:, IB:2 * IB].bitcast(f16)
                        else:
                            p16 = p16_pool.tile([P, 2 * IB], f16, tag="p16")
                            nc.scalar.activation(p16[:], s[:], Exp,
                                                 scale=float(1.0 / A_H),
                                                 bias=bias_c[:])
                            rhs0 = p16[:, 0:IB]
                            rhs1 = p16[:, IB:2 * IB]

                        def make_pv(rhs0=rhs0, rhs1=rhs1, c0=c0, c1=c1, t=t,
                                    acc=acc, v16=v16, first=first):
                            def pv():
                                nc.tensor.matmul(
                                    acc[:], lhsT=v16[:, c0, :], rhs=rhs0,
                                    start=first[0], stop=False)
                                first[0] = False
                                nc.tensor.matmul(
                                    acc[:], lhsT=v16[:, c1, :], rhs=rhs1,
                                    start=False, stop=(t == npair - 1))
                            return pv

                        pending_pvs.append(make_pv())
                        if len(pending_pvs) > 3:
                            pending_pvs.pop(0)()
                        if t == 4 and pending_out is not None:
                            pending_out()
                            pending_out = None

                    def make_out(acc=acc, h=h, ib=ib):
                        def out():
                            # 128-partition staging keeps the DMA engine
                            # fan-out balanced (rows 97..127 are junk)
                            o_sb = out_pool.tile([P, IB], f16, tag="osb",
                                                 name=f"o_{h}_{ib}")
                            nc.vector.tensor_copy(o_sb[0:97, :],
                                                  acc[0:97, :])
                            nc.sync.dma_start(o_d[h, ib], o_sb[:])
                        return out

                    if pending_out is not None:
                        pending_out()
                    pending_out = make_out()

            while pending_pvs:
                pending_pvs.pop(0)()
            if pending_out is not None:
                pending_out()

    nc.compile()
    return nc


def kernel(hidden_states, q, k, v, bboxes, is_cross, ith, num_heads):
    global LAST_RESULTS
    if is_cross:
        return np.asarray(hidden_states)

    from concourse.bass_utils import run_bass_kernel_spmd

    q = np.asarray(q, dtype=np.float32)
    k = np.asarray(k, dtype=np.float32)
    v = np.asarray(v, dtype=np.float32)
    bboxes = np.asarray(bboxes, dtype=np.float32)
    num_heads = int(num_heads)

    bh, n, dh = q.shape
    assert dh == DH and bh % N_CORES == 0 and n % IB == 0
    heads_per_core = bh // N_CORES
    batch = bh // num_heads
    nch = n // P
    n_ib = n // IB
    scale = 1.0 / math.sqrt(dh)
    g = math.sqrt(A_H * scale)

    res_sq = int(math.isqrt(n))
    subj = _subject_masks_np(bboxes, res_sq)
    assert subj.shape[0] == 2, "kernel specialized for 2 subject boxes"
    mA = (subj[0] & ~subj[1]).astype(np.float32)  # A-only
    mB = (subj[1] & ~subj[0]).astype(np.float32)  # B-only

    f16 = np.float16

    qT = q.transpose(0, 2, 1) * g  # [bh, 80, n]
    kT = k.transpose(0, 2, 1) * g
    q16 = np.zeros((bh, P, n), f16)
    q16[:, :DH] = qT
    q16[:, DH] = MASK * mA
    q16[:, DH + 1] = MASK * mB
    k16 = np.zeros((bh, P, n), f16)
    k16[:, :DH] = kT
    k16[:, DH] = -MASK * mB
    k16[:, DH + 1] = -MASK * mA
    k16 = k16.reshape(bh, P, nch, P)

    vt = np.zeros((bh, n, DV), f16)
    vt[:, :, :DH] = v
    vt[:, :, SUM_ROW] = 1.0
    v16 = np.ascontiguousarray(
        vt.reshape(bh, nch, P, DV).transpose(0, 2, 1, 3))

    key = (n, heads_per_core, DVE_PAIRS)
    if key not in _PROGRAM_CACHE:
        _PROGRAM_CACHE[key] = _build_program(n, heads_per_core,
                                             frozenset(DVE_PAIRS))
    nc = _PROGRAM_CACHE[key]

    in_maps = []
    for c in range(N_CORES):
        sl = slice(c * heads_per_core, (c + 1) * heads_per_core)
        in_maps.append({"q16": q16[sl], "k16": k16[sl], "v16": v16[sl]})

    trace = bool(int(os.environ.get("BASS_ATTN_TRACE", "0")))
    kwargs = {}
    if trace:
        kwargs = dict(trace=True, trace_cores=list(range(N_CORES)))
    res = run_bass_kernel_spmd(nc, in_maps, core_ids=list(range(N_CORES)),
                               **kwargs)
    LAST_RESULTS = res

    out = np.empty((batch, n, num_heads * dh), np.float32)
    for bh_idx in range(bh):
        c, hh = divmod(bh_idx, heads_per_core)
        b, hd = divmod(bh_idx, num_heads)
        o = res.results[c]["o"][hh][:, 0:97].astype(np.float32)
        den = o[:, SUM_ROW, :]  # [n_ib, IB]
        on = o[:, :DH, :] / den[:, None, :]  # [n_ib, 80, IB]
        out[b, :, hd * dh:(hd + 1) * dh] = (
            on.transpose(0, 2, 1).reshape(n, dh))
    return out
